# revision 1
# baseline (speedup 1.0000x reference)
"""Trainium2 Bass kernel for nn_GResBlock (2-layer weighted-GCN residual block).

    h1 = relu(A @ x @ W1 + x @ W1_loop + b1)
    h2 = relu(A @ h1 @ W2 + h1 @ W2_loop + b2)
    out = (x + h2) * 0.5
(A = 50000^2 sparse adjacency given as an 800000-edge weighted list.)

Strategy (8 NeuronCores, SPMD — one program, per-core data):
- Vertices padded to 50176 = 8*6272 rows; core c owns dst nodes
  [c*6272, (c+1)*6272) split into 98 chunks of 64. Edges are bucketed by
  dst core, sorted by dst chunk, and split by src < 32768 (lo) / >= (hi)
  so int16 dma_gather indices stay in range (hi calls use a shifted base).
- Aggregation is reordered as (A @ x) @ W (associativity), so the gather
  table for layer 1 is x itself (bf16, rows padded to 256B) — no support
  matrix is ever materialized.
- Per chunk, each 128-edge block is one PE matmul: stationary = gathered
  src rows [128, 96], moving = a host-built block-sparse selector
  S [128, 64] (edge weights at the edge's dst lane; zero rows for padding),
  accumulating agg^T [96, 64] f32 in PSUM. Edge weights ride in S for free.
- Then psum2 = Wloop_aug^T @ src_aug (bias folded via an ones row) +
  W^T @ agg (one more matmul each) -> relu -> h^T chunk.
- Layer 1 tail: PE-transpose each h1^T chunk -> h1 rows -> internal DRAM;
  one AllGather builds the full 50176-row layer-2 gather table.
- Layer 2 tail: out^T = x^T/2 + relu(psum2 * 0.5); output returned
  transposed per core and re-assembled on the host.
- Gathers are 1024-index dma_gather calls (hardware scratch cap) spread
  round-robin over 4 SWDGE queues; explicit order deps keep Tile's DMASW
  sem-lane rotation consistent with the queue rotation.
"""
import os
import sys

import numpy as np
import ml_dtypes

try:
    import concourse.bass  # noqa: F401
except ImportError:
    sys.path.insert(0, "/opt/trn_rl_repo")

import concourse.bass as bass  # noqa: E402
import concourse.tile as tile  # noqa: E402
from concourse.tile_rust import add_dep_helper  # noqa: E402
from concourse import bacc, mybir  # noqa: E402
from concourse.library_config import mlp  # noqa: E402
from concourse.bass_utils import run_bass_kernel_spmd  # noqa: E402

bf16 = ml_dtypes.bfloat16
BF16 = mybir.dt.bfloat16
F32 = mybir.dt.float32
I16 = mybir.dt.int16

N_NODES = 50000
D = 96
NC = 8
SHARD = 6272
NPAD = NC * SHARD          # 50176
CHUNK = 64
NCHUNK = SHARD // CHUNK    # 98
HALF = 32768
ELEM = 128                 # gather element width (bf16 -> 256B)
NQ = 4                     # SWDGE queues
CALL_BLK = 8               # 128-edge blocks per gather call
CALL_IDX = 1024            # indices per gather call (hw scratch cap)


def _wrap_idx(idx):
    """[n] -> [128, n//16] int16 wrapped layout (idx i at [i%16, i//16],
    replicated across the 8 16-partition groups)."""
    n = idx.shape[0]
    w16 = idx.reshape(n // 16, 16).T.astype(np.int16)
    return np.tile(w16, (8, 1))


def _preprocess(edge_src, edge_dst, edge_weight):
    edge_src = np.asarray(edge_src).astype(np.int64)
    edge_dst = np.asarray(edge_dst).astype(np.int64)
    edge_weight = np.asarray(edge_weight).astype(np.float32)

    core_of = edge_dst // SHARD
    percore = []
    n_lo = np.zeros((NC, NCHUNK), np.int64)
    n_hi = np.zeros((NC, NCHUNK), np.int64)
    for c in range(NC):
        m = core_of == c
        s, d, w = edge_src[m], edge_dst[m], edge_weight[m]
        dl = d - c * SHARD
        ch = dl // CHUNK
        lane = dl % CHUNK
        lo = s < HALF
        order = np.lexsort((np.arange(len(s)), ~lo, ch))
        s, ch, lane, w, lo = s[order], ch[order], lane[order], w[order], lo[order]
        percore.append((s, ch, lane, w, lo))
        for k in range(NCHUNK):
            mk = ch == k
            n_lo[c, k] = np.count_nonzero(mk & lo)
            n_hi[c, k] = np.count_nonzero(mk & ~lo)

    B_lo = max(1, int(np.ceil(n_lo.max() / 128)))
    B_hi = max(1, int(np.ceil(n_hi.max() / 128)))
    NB = B_lo + B_hi

    out = []
    for c in range(NC):
        s, ch, lane, w, lo = percore[c]
        idx_lo = np.zeros((NCHUNK, B_lo * 128), np.int64)
        idx_hi = np.zeros((NCHUNK, B_hi * 128), np.int64)
        S = np.zeros((NCHUNK, 128, NB * CHUNK), np.float32)
        for k in range(NCHUNK):
            mk = ch == k
            sl, lal, wl, lol = s[mk], lane[mk], w[mk], lo[mk]
            a_s, a_l, a_w = sl[lol], lal[lol], wl[lol]
            nb = len(a_s)
            idx_lo[k, :nb] = a_s
            pos = np.arange(nb)
            S[k, pos % 128, (pos // 128) * CHUNK + a_l] = a_w
            b_s, b_l, b_w = sl[~lol], lal[~lol], wl[~lol]
            nb = len(b_s)
            idx_hi[k, :nb] = b_s - HALF
            pos = np.arange(nb)
            S[k, pos % 128, (B_lo + pos // 128) * CHUNK + b_l] = b_w

        def to_calls(idx2d, B):
            flat = idx2d.reshape(NCHUNK * B * 128)
            ncall = -(-(NCHUNK * B) // CALL_BLK)
            flat = np.concatenate([flat, np.zeros(ncall * CALL_IDX - flat.shape[0], np.int64)])
            return np.stack([_wrap_idx(flat[i * CALL_IDX:(i + 1) * CALL_IDX])
                             for i in range(ncall)])

        out.append(dict(
            idx_lo=to_calls(idx_lo, B_lo).astype(np.int16),
            idx_hi=to_calls(idx_hi, B_hi).astype(np.int16),
            S=S.astype(bf16),
        ))
    return out, B_lo, B_hi


def _make_in_maps(x, W1, W1_loop, b1, W2, W2_loop, b2, edge_weight, edge_src, edge_dst):
    pp, B_lo, B_hi = _preprocess(edge_src, edge_dst, edge_weight)
    x = np.asarray(x, np.float32)
    xtab = np.zeros((NPAD, ELEM), bf16)
    xtab[:N_NODES, :D] = x.astype(bf16)
    xpad = np.zeros((NPAD, D), np.float32)
    xpad[:N_NODES] = x
    W1a = np.concatenate([np.asarray(W1_loop, np.float32),
                          np.asarray(b1, np.float32)[None, :]], 0).astype(bf16)
    W2a = np.concatenate([np.asarray(W2_loop, np.float32),
                          np.asarray(b2, np.float32)[None, :]], 0).astype(bf16)
    in_maps = []
    for c in range(NC):
        xs = xpad[c * SHARD:(c + 1) * SHARD]
        xT_aug = np.ones((D + 1, SHARD), bf16)
        xT_aug[:D] = xs.T.astype(bf16)
        in_maps.append(dict(
            xtab=xtab,
            xT_aug=xT_aug,
            xT_half=np.ascontiguousarray(0.5 * xs.T).astype(np.float32),
            W1=np.asarray(W1, np.float32).astype(bf16),
            W2=np.asarray(W2, np.float32).astype(bf16),
            W1a=W1a, W2a=W2a,
            S=pp[c]["S"],
            idx_lo=pp[c]["idx_lo"],
            idx_hi=pp[c]["idx_hi"],
        ))
    return in_maps, B_lo, B_hi


def build_program(B_lo, B_hi, repeat=0, ag_reps=1, parts="all"):
    """Build the SPMD Bass program. repeat>0 wraps each gconv phase in a
    hardware For_i loop and emits the AllGather ag_reps times (timing only;
    collectives cannot sit inside hardware loops)."""
    NB = B_lo + B_hi
    NCALL_LO = -(-(NCHUNK * B_lo) // CALL_BLK)
    NCALL_HI = -(-(NCHUNK * B_hi) // CALL_BLK)
    nc = bacc.Bacc("TRN2", target_bir_lowering=False, debug=False, num_devices=NC,
                   num_swdge_queues=NQ)

    xtab = nc.dram_tensor("xtab", [NPAD, ELEM], BF16, kind="ExternalInput")
    xT_aug = nc.dram_tensor("xT_aug", [D + 1, SHARD], BF16, kind="ExternalInput")
    xT_half = nc.dram_tensor("xT_half", [D, SHARD], F32, kind="ExternalInput")
    W1 = nc.dram_tensor("W1", [D, D], BF16, kind="ExternalInput")
    W2 = nc.dram_tensor("W2", [D, D], BF16, kind="ExternalInput")
    W1a = nc.dram_tensor("W1a", [D + 1, D], BF16, kind="ExternalInput")
    W2a = nc.dram_tensor("W2a", [D + 1, D], BF16, kind="ExternalInput")
    S_d = nc.dram_tensor("S", [NCHUNK, 128, NB * CHUNK], BF16, kind="ExternalInput")
    idx_lo_d = nc.dram_tensor("idx_lo", [NCALL_LO, 128, CALL_IDX // 16], I16,
                              kind="ExternalInput")
    idx_hi_d = nc.dram_tensor("idx_hi", [NCALL_HI, 128, CALL_IDX // 16], I16,
                              kind="ExternalInput")
    outT = nc.dram_tensor("outT", [D, SHARD], F32, kind="ExternalOutput")

    with tile.TileContext(nc) as tc:
        from contextlib import ExitStack
        with ExitStack() as ctx:
            const = ctx.enter_context(tc.tile_pool(name="const", bufs=1))
            idxp = ctx.enter_context(tc.tile_pool(name="idxp", bufs=6))
            mlop = ctx.enter_context(tc.tile_pool(name="mlop", bufs=6))
            mhip = ctx.enter_context(tc.tile_pool(name="mhip", bufs=4))
            sp = ctx.enter_context(tc.tile_pool(name="sp", bufs=3))
            srcp = ctx.enter_context(tc.tile_pool(name="srcp", bufs=3))
            aggsbp = ctx.enter_context(tc.tile_pool(name="aggsbp", bufs=3))
            rowp = ctx.enter_context(tc.tile_pool(name="rowp", bufs=3))
            outp = ctx.enter_context(tc.tile_pool(name="outp", bufs=3))
            aggps = ctx.enter_context(tc.tile_pool(name="aggps", bufs=3, space="PSUM"))
            p2ps = ctx.enter_context(tc.tile_pool(name="p2ps", bufs=2, space="PSUM"))
            trps = ctx.enter_context(tc.tile_pool(name="trps", bufs=2, space="PSUM"))

            nc.gpsimd.load_library(mlp)

            ident_d = nc.inline_tensor(np.eye(D, dtype=bf16), name="ident_bf16")
            ident = const.tile([D, D], BF16)
            nc.sync.dma_start(ident[:], ident_d.ap())
            w1 = const.tile([D, D], BF16)
            nc.sync.dma_start(w1[:], W1.ap())
            w2 = const.tile([D, D], BF16)
            nc.sync.dma_start(w2[:], W2.ap())
            w1a = const.tile([D + 1, D], BF16)
            nc.sync.dma_start(w1a[:], W1a.ap())
            w2a = const.tile([D + 1, D], BF16)
            nc.sync.dma_start(w2a[:], W2a.ap())

            h1t = const.tile([D + 1, SHARD], BF16)   # persistent h1^T (+ones row)
            nc.vector.memset(h1t[D:D + 1, :], 1.0)

            state = {"gq": 0, "prev_gather": None}
            h1_local = nc.dram_tensor("h1_local", [SHARD, ELEM], BF16, kind="Internal").ap()
            h1_table = nc.dram_tensor("h1_table", [NPAD, ELEM], BF16, kind="Internal",
                                      addr_space="Shared").ap()

            def gconv(layer, table_ap, w_t, wa_t):
                lo_tiles = {}
                hi_tiles = {}

                NOHI = os.environ.get("GK_NOHI", "0") == "1"
                NOIDX = os.environ.get("GK_NOIDX", "0") == "1"

                def emit_call(tiles, idx_d, c, half):
                    if half == 1 and NOHI:
                        tiles[c] = None
                        return
                    it = idxp.tile([128, CALL_IDX // 16], I16, tag="it")
                    nc.sync.dma_start(it[:], idx_d.ap()[0] if NOIDX else idx_d.ap()[c])
                    m = (mlop if half == 0 else mhip).tile(
                        [128, CALL_BLK, ELEM], BF16, tag="m")
                    base = table_ap[0:HALF, :] if half == 0 else table_ap[HALF:NPAD, :]
                    if parts == "nogather":
                        nc.vector.memset(m[:, 0:1, :], 0.0)
                        tiles[c] = m
                        return
                    gi = nc.gpsimd.dma_gather(m[:], base, it[:], CALL_IDX, CALL_IDX,
                                              ELEM, queue_num=state["gq"] % NQ)
                    state["gq"] += 1
                    if state["prev_gather"] is not None:
                        # Keep Pool-engine order = emission order so Tile's
                        # 8-lane DMASW sem rotation stays aligned with the
                        # 4-queue rotation (sems are queue-locked).
                        add_dep_helper(gi.ins, state["prev_gather"].ins, sync=False,
                                       reason="swdge queue/sem-lane consistency")
                    state["prev_gather"] = gi
                    tiles[c] = m

                for k in range(NCHUNK):
                    for j in range(B_lo):
                        c = (k * B_lo + j) // CALL_BLK
                        if c not in lo_tiles:
                            emit_call(lo_tiles, idx_lo_d, c, 0)
                    for j in range(B_hi):
                        c = (k * B_hi + j) // CALL_BLK
                        if c not in hi_tiles:
                            emit_call(hi_tiles, idx_hi_d, c, 1)
                    if parts in ("gather", "g1"):
                        continue
                    st = sp.tile([128, NB * CHUNK], BF16, tag="st")
                    nc.sync.dma_start(st[:], S_d.ap()[k])
                    agg = aggps.tile([D, CHUNK], F32, tag="agg")
                    for j in range(B_lo):
                        b = k * B_lo + j
                        nc.tensor.matmul(
                            agg[:], lo_tiles[b // CALL_BLK][:, b % CALL_BLK, 0:D],
                            st[:, j * CHUNK:(j + 1) * CHUNK],
                            start=(j == 0), stop=False, skip_group_check=True)
                    for j in range(B_hi):
                        b = k * B_hi + j
                        nc.tensor.matmul(
                            agg[:], hi_tiles[b // CALL_BLK][:, b % CALL_BLK, 0:D],
                            st[:, (B_lo + j) * CHUNK:(B_lo + j + 1) * CHUNK],
                            start=False, stop=(j == B_hi - 1), skip_group_check=True)
                    aggb = aggsbp.tile([D, CHUNK], BF16, tag="aggb")
                    nc.scalar.activation(aggb[:], agg[:],
                                         mybir.ActivationFunctionType.Copy)
                    p2 = p2ps.tile([D, CHUNK], F32, tag="p2")
                    if layer == 1:
                        src = srcp.tile([D + 1, CHUNK], BF16, tag="src")
                        nc.sync.dma_start(src[:], xT_aug.ap()[:, k * CHUNK:(k + 1) * CHUNK])
                        srcap = src[:]
                    else:
                        srcap = h1t[:, k * CHUNK:(k + 1) * CHUNK]
                    nc.tensor.matmul(p2[:], wa_t[:], srcap,
                                     start=True, stop=False, skip_group_check=True)
                    nc.tensor.matmul(p2[:], w_t[:], aggb[:],
                                     start=False, stop=True, skip_group_check=True)
                    if layer == 1:
                        hslice = h1t[0:D, k * CHUNK:(k + 1) * CHUNK]
                        nc.scalar.activation(hslice, p2[:],
                                             mybir.ActivationFunctionType.Relu)
                        trp = trps.tile([CHUNK, D], BF16, tag="trp")
                        nc.tensor.transpose(trp[:], hslice, ident[:])
                        row = rowp.tile([CHUNK, D], BF16, tag="row")
                        nc.vector.tensor_copy(row[:], trp[:])
                        nc.sync.dma_start(h1_local[k * CHUNK:(k + 1) * CHUNK, 0:D], row[:])
                    else:
                        relu = outp.tile([D, CHUNK], F32, tag="relu")
                        nc.scalar.activation(relu[:], p2[:],
                                             mybir.ActivationFunctionType.Relu, scale=0.5)
                        xh = srcp.tile([D, CHUNK], F32, tag="xh")
                        nc.sync.dma_start(xh[:], xT_half.ap()[:, k * CHUNK:(k + 1) * CHUNK])
                        ot = outp.tile([D, CHUNK], F32, tag="ot")
                        nc.vector.tensor_add(ot[:], relu[:], xh[:])
                        nc.sync.dma_start(outT.ap()[:, k * CHUNK:(k + 1) * CHUNK], ot[:])

            if parts == "gather":
                zt = outp.tile([D, SHARD], F32, tag="zt")
                nc.vector.memset(zt[:], 0.0)
                nc.sync.dma_start(outT.ap(), zt[:])

            def allgather():
                nc.gpsimd.collective_compute(
                    "AllGather", mybir.AluOpType.bypass,
                    ins=[h1_local[:]], outs=[h1_table[:]],
                    replica_groups=[list(range(NC))],
                )

            if repeat > 0 and parts == "g1":
                with tc.For_i(0, repeat, 1):
                    gconv(1, xtab.ap(), w1, w1a)
                zt = outp.tile([D, SHARD], F32, tag="zt2")
                nc.vector.memset(zt[:], 0.0)
                nc.sync.dma_start(outT.ap(), zt[:])
            elif repeat > 0:
                with tc.For_i(0, repeat, 1):
                    gconv(1, xtab.ap(), w1, w1a)
                state["prev_gather"] = None
                for _ in range(ag_reps):
                    allgather()
                with tc.For_i(0, repeat, 1):
                    gconv(2, h1_table[:], w2, w2a)
            else:
                gconv(1, xtab.ap(), w1, w1a)
                allgather()
                gconv(2, h1_table[:], w2, w2a)

    nc.compile()
    return nc


_CACHE = {}


def kernel(**inputs):
    in_maps, B_lo, B_hi = _make_in_maps(**inputs)
    key = (B_lo, B_hi)
    if key not in _CACHE:
        _CACHE[key] = build_program(B_lo, B_hi)
    nc = _CACHE[key]
    r = run_bass_kernel_spmd(nc, in_maps, list(range(NC)))
    out = np.concatenate([r.results[c]["outT"].T for c in range(NC)], 0)[:N_NODES]
    return np.ascontiguousarray(out.astype(np.float32))



# revision 32
# speedup vs baseline: 24.7919x; 24.7919x over previous
"""Trainium2 Bass kernel for nn_GResBlock (2-layer weighted-GCN residual block).

    h1 = relu(A @ x @ W1 + x @ W1_loop + b1)
    h2 = relu(A @ h1 @ W2 + h1 @ W2_loop + b2)
    out = (x + h2) * 0.5
(A = 50000^2 sparse adjacency given as an 800000-edge weighted list.)

Strategy (8 NeuronCores, SPMD — one program, per-core data):
- Vertices padded to 50176 = 8*6272 rows; core c owns dst nodes
  [c*6272, (c+1)*6272) split into 98 chunks of 64. Edges are bucketed by
  dst core, sorted by dst chunk, and split by src < 32768 (lo) / >= (hi)
  so int16 dma_gather indices stay in range (hi calls use a shifted base).
- Aggregation is reordered as (A @ x) @ W (associativity), so the gather
  table for layer 1 is x itself (bf16, rows padded to 256B) — no support
  matrix is ever materialized.
- Per chunk, each 128-edge block is one PE matmul: stationary = gathered
  src rows [128, 96], moving = a host-built block-sparse selector
  S [128, 64] (edge weights at the edge's dst lane; zero rows for padding),
  accumulating agg^T [96, 64] f32 in PSUM. Edge weights ride in S for free.
- Then psum2 = Wloop_aug^T @ src_aug (bias folded via an ones row) +
  W^T @ agg (one more matmul each) -> relu -> h^T chunk.
- Layer 1 tail: PE-transpose each h1^T chunk -> h1 rows -> internal DRAM;
  one AllGather builds the full 50176-row layer-2 gather table.
- Layer 2 tail: out^T = x^T/2 + relu(psum2 * 0.5); output returned
  transposed per core and re-assembled on the host.
- Gathers are 1024-index dma_gather calls (hardware scratch cap) spread
  round-robin over 4 SWDGE queues; explicit order deps keep Tile's DMASW
  sem-lane rotation consistent with the queue rotation.
"""
import os
import sys

import numpy as np
import ml_dtypes

try:
    import concourse.bass  # noqa: F401
except ImportError:
    sys.path.insert(0, "/opt/trn_rl_repo")

import concourse.bass as bass  # noqa: E402
import concourse.tile as tile  # noqa: E402
from concourse.tile_rust import add_dep_helper  # noqa: E402
from concourse import bacc, mybir  # noqa: E402
from concourse.library_config import mlp  # noqa: E402
from concourse.bass_utils import run_bass_kernel_spmd  # noqa: E402

bf16 = ml_dtypes.bfloat16
BF16 = mybir.dt.bfloat16
F32 = mybir.dt.float32
I16 = mybir.dt.int16

N_NODES = 50000
D = 96
NC = 8
SHARD = 6272
NPAD = NC * SHARD          # 50176
CHUNK = 64
NCHUNK = SHARD // CHUNK    # 98
HALF = 32768
ELEM = 128                 # gather element width (bf16 -> 256B)
NQ = 4                     # SWDGE queues
CALL_BLK = 8               # 128-edge blocks per gather call
CALL_IDX = 1024            # indices per gather call (hw scratch cap)


def _wrap_idx(idx):
    """[n] -> [128, n//16] int16 wrapped layout (idx i at [i%16, i//16],
    replicated across the 8 16-partition groups)."""
    n = idx.shape[0]
    w16 = idx.reshape(n // 16, 16).T.astype(np.int16)
    return np.tile(w16, (8, 1))


def _preprocess(edge_src, edge_dst, edge_weight, call_blk=CALL_BLK):
    CALL_BLK = call_blk
    CALL_IDX = call_blk * 128
    edge_src = np.asarray(edge_src).astype(np.int64)
    edge_dst = np.asarray(edge_dst).astype(np.int64)
    edge_weight = np.asarray(edge_weight).astype(np.float32)

    core_of = edge_dst // SHARD
    percore = []
    n_lo = np.zeros((NC, NCHUNK), np.int64)
    n_hi = np.zeros((NC, NCHUNK), np.int64)
    for c in range(NC):
        m = core_of == c
        s, d, w = edge_src[m], edge_dst[m], edge_weight[m]
        dl = d - c * SHARD
        ch = dl // CHUNK
        lane = dl % CHUNK
        lo = s < HALF
        order = np.lexsort((np.arange(len(s)), ~lo, ch))
        s, ch, lane, w, lo = s[order], ch[order], lane[order], w[order], lo[order]
        percore.append((s, ch, lane, w, lo))
        for k in range(NCHUNK):
            mk = ch == k
            n_lo[c, k] = np.count_nonzero(mk & lo)
            n_hi[c, k] = np.count_nonzero(mk & ~lo)

    B_lo = max(1, int(np.ceil(n_lo.max() / 128)))
    B_hi = max(1, int(np.ceil(n_hi.max() / 128)))
    NB = B_lo + B_hi

    out = []
    for c in range(NC):
        s, ch, lane, w, lo = percore[c]
        idx_lo = np.zeros((NCHUNK, B_lo * 128), np.int64)
        idx_hi = np.zeros((NCHUNK, B_hi * 128), np.int64)
        S = np.zeros((NCHUNK, 128, NB * CHUNK), np.float32)
        for k in range(NCHUNK):
            mk = ch == k
            sl, lal, wl, lol = s[mk], lane[mk], w[mk], lo[mk]
            a_s, a_l, a_w = sl[lol], lal[lol], wl[lol]
            nb = len(a_s)
            idx_lo[k, :nb] = a_s
            pos = np.arange(nb)
            S[k, pos % 128, (pos // 128) * CHUNK + a_l] = a_w
            b_s, b_l, b_w = sl[~lol], lal[~lol], wl[~lol]
            nb = len(b_s)
            idx_hi[k, :nb] = b_s - HALF
            pos = np.arange(nb)
            S[k, pos % 128, (B_lo + pos // 128) * CHUNK + b_l] = b_w

        def to_calls(idx2d, B):
            flat = idx2d.reshape(NCHUNK * B * 128)
            ncall = -(-(NCHUNK * B) // CALL_BLK)
            flat = np.concatenate([flat, np.zeros(ncall * CALL_IDX - flat.shape[0], np.int64)])
            return np.stack([_wrap_idx(flat[i * CALL_IDX:(i + 1) * CALL_IDX])
                             for i in range(ncall)])

        out.append(dict(
            idx_lo=to_calls(idx_lo, B_lo).astype(np.int16),
            idx_hi=to_calls(idx_hi, B_hi).astype(np.int16),
            S=S.astype(bf16),
        ))
    return out, B_lo, B_hi


def _make_in_maps(x, W1, W1_loop, b1, W2, W2_loop, b2, edge_weight, edge_src, edge_dst,
                  elem=ELEM, call_blk=CALL_BLK):
    ELEM = elem
    pp, B_lo, B_hi = _preprocess(edge_src, edge_dst, edge_weight, call_blk=call_blk)
    x = np.asarray(x, np.float32)
    xtab = np.zeros((NPAD, ELEM), bf16)
    xtab[:N_NODES, :D] = x.astype(bf16)
    xpad = np.zeros((NPAD, D), np.float32)
    xpad[:N_NODES] = x
    W1a = np.concatenate([np.asarray(W1_loop, np.float32),
                          np.asarray(b1, np.float32)[None, :]], 0).astype(bf16)
    W2a = np.concatenate([np.asarray(W2_loop, np.float32),
                          np.asarray(b2, np.float32)[None, :]], 0).astype(bf16)
    in_maps = []
    for c in range(NC):
        xs = xpad[c * SHARD:(c + 1) * SHARD]
        xT_aug = np.ones((D + 1, SHARD), bf16)
        xT_aug[:D] = xs.T.astype(bf16)
        in_maps.append(dict(
            xtab=xtab,
            xT_aug=xT_aug,
            xT_half=np.ascontiguousarray(0.5 * xs.T).astype(np.float32),
            W1=np.asarray(W1, np.float32).astype(bf16),
            W2=np.asarray(W2, np.float32).astype(bf16),
            W1a=W1a, W2a=W2a,
            S=pp[c]["S"],
            idx_lo=pp[c]["idx_lo"],
            idx_hi=pp[c]["idx_hi"],
        ))
    return in_maps, B_lo, B_hi


def _balance_perm(edge_src, edge_dst):
    """Assign dst nodes to (core, chunk, lane) slots so that every
    (core, chunk) bin has near-equal lo/hi in-edge counts.  Greedy
    largest-degree-first into the currently-lightest bin.  Returns pos[node]
    = permuted position.  Everything (x table, h1 table, output) uses
    pos-order so both layers share one stream structure."""
    nbin = NC * NCHUNK
    lo_deg = np.bincount(edge_dst[edge_src < HALF], minlength=N_NODES)[:N_NODES]
    hi_deg = np.bincount(edge_dst[edge_src >= HALF], minlength=N_NODES)[:N_NODES]
    tot = lo_deg + hi_deg
    order = np.argsort(-tot, kind="stable")
    lo_sum = np.zeros(nbin, np.int64)
    hi_sum = np.zeros(nbin, np.int64)
    n_in = np.zeros(nbin, np.int64)
    mu_lo = max(lo_deg.sum() / nbin, 1.0)
    mu_hi = max(hi_deg.sum() / nbin, 1.0)
    assign = np.empty(N_NODES, np.int64)
    # vectorized greedy: process nodes in batches of one per bin pass
    full = np.zeros(nbin, bool)
    for v in order:
        cost = np.maximum((lo_sum + lo_deg[v]) / mu_lo,
                          (hi_sum + hi_deg[v]) / mu_hi)
        cost[full] = np.inf
        b = int(np.argmin(cost))
        assign[v] = b
        lo_sum[b] += lo_deg[v]
        hi_sum[b] += hi_deg[v]
        n_in[b] += 1
        if n_in[b] == CHUNK:
            full[b] = True
    # positions: nodes of bin b occupy [b*CHUNK, b*CHUNK + n_in[b])
    pos = np.empty(NPAD, np.int64)
    fill = np.zeros(nbin, np.int64)
    for v in range(N_NODES):
        b = assign[v]
        pos[v] = b * CHUNK + fill[b]
        fill[b] += 1
    # pad nodes fill the remaining lanes
    spare = NPAD - N_NODES
    holes = []
    for b in range(nbin):
        holes.extend(range(b * CHUNK + fill[b], (b + 1) * CHUNK))
    assert len(holes) == spare
    pos[N_NODES:] = np.array(holes, np.int64)
    return pos


def _preprocess2(edge_src, edge_dst, edge_weight, balance=True,
                 call_blk=CALL_BLK):
    CALL_BLK = call_blk
    CALL_IDX = call_blk * 128
    """v2 layout: per stream (lo/hi), chunks occupy core-invariant slot
    windows of width max-over-cores count (~8% padding vs 25% for the v1
    per-chunk 128-rounded max).  128-slot blocks and 1024-slot gather calls
    tile each stream across chunk boundaries; the matmul schedule is the
    static pair list (block, chunk) derived from the slot layout.
    With balance=True, dst nodes are first permuted into (core, chunk) bins
    with near-equal counts, shrinking windows to ~the mean (#slots -> floor).
    NOTE: the lo/hi stream split keys on the PERMUTED src position."""
    edge_src = np.asarray(edge_src).astype(np.int64)
    edge_dst = np.asarray(edge_dst).astype(np.int64)
    edge_weight = np.asarray(edge_weight).astype(np.float32)
    if balance:
        pos = _balance_perm(edge_src, edge_dst)
        edge_src = pos[edge_src]
        edge_dst = pos[edge_dst]
    else:
        pos = np.arange(NPAD, dtype=np.int64)

    # per core / stream / chunk edge lists (stable order)
    percore = []
    cnt = np.zeros((NC, 2, NCHUNK), np.int64)
    for c in range(NC):
        m = (edge_dst // SHARD) == c
        s, d, w = edge_src[m], edge_dst[m], edge_weight[m]
        dl = d - c * SHARD
        ch = dl // CHUNK
        lane = dl % CHUNK
        hi = (s >= HALF).astype(np.int64)
        order = np.lexsort((np.arange(len(s)), hi, ch))
        s, ch, lane, w, hi = s[order], ch[order], lane[order], w[order], hi[order]
        percore.append((s, ch, lane, w, hi))
        for t in range(2):
            cnt[c, t] = np.bincount(ch[hi == t], minlength=NCHUNK)

    w_tk = cnt.max(axis=0)                     # [2, NCHUNK] slot window widths
    off = np.zeros((2, NCHUNK + 1), np.int64)  # cumulative slot offsets
    off[:, 1:] = np.cumsum(w_tk, axis=1)
    nslot = off[:, -1]
    ncall = [int(-(-nslot[t] // CALL_IDX)) for t in range(2)]
    nslot_pad = [ncall[t] * CALL_IDX for t in range(2)]

    # static matmul pair list: per chunk, lo pairs then hi pairs
    pairs = []            # (t, b, k)
    chunk_pairs = []      # per chunk: (start_pair_idx, npairs)
    for k in range(NCHUNK):
        p0 = len(pairs)
        for t in range(2):
            b0 = off[t, k] // 128
            b1 = (off[t, k + 1] - 1) // 128
            for b in range(b0, b1 + 1):
                pairs.append((t, b, k))
        chunk_pairs.append((p0, len(pairs) - p0))
    npair = len(pairs)

    out = []
    for c in range(NC):
        s, ch, lane, w, hi = percore[c]
        idx_flat = [np.zeros(nslot_pad[t], np.int64) for t in range(2)]
        S = np.zeros((128, 64 * npair), np.float32)
        # slot position of each edge: stream offset of its chunk + rank in chunk
        for t in range(2):
            mt = hi == t
            st_ch, st_lane, st_w, st_s = ch[mt], lane[mt], w[mt], s[mt]
            # rank within (chunk) in stable order
            rank = np.zeros(len(st_ch), np.int64)
            for k in range(NCHUNK):
                mk = st_ch == k
                rank[mk] = np.arange(np.count_nonzero(mk))
            slot = off[t, st_ch] + rank
            idx_flat[t][slot] = st_s - t * HALF
            blk = slot // 128
            # pair index of (t, blk, chunk): build lookup
            # pairs are ordered; map (t,b,k) -> pair idx
            pair_of = {pr: i for i, pr in enumerate(pairs)}
            pidx = np.array([pair_of[(t, int(b), int(k))]
                             for b, k in zip(blk, st_ch)], np.int64)
            S[slot % 128, 64 * pidx + st_lane] = st_w

        def to_calls(flat, n):
            return np.stack([_wrap_idx(flat[i * CALL_IDX:(i + 1) * CALL_IDX])
                             for i in range(n)])

        out.append(dict(
            idx_lo=to_calls(idx_flat[0], ncall[0]).astype(np.int16),
            idx_hi=to_calls(idx_flat[1], ncall[1]).astype(np.int16),
            S=S.astype(bf16),
        ))
    layout = dict(off=off, ncall=ncall, pairs=pairs, chunk_pairs=chunk_pairs,
                  npair=npair, pos=pos, call_blk=call_blk)
    return out, layout


def _make_in_maps2(x, W1, W1_loop, b1, W2, W2_loop, b2, edge_weight, edge_src,
                   edge_dst, call_blk=CALL_BLK):
    pp, layout = _preprocess2(edge_src, edge_dst, edge_weight, call_blk=call_blk)
    pos = layout["pos"]
    x = np.asarray(x, np.float32)
    xtab = np.zeros((NPAD, ELEM), bf16)
    xtab[pos[:N_NODES], :D] = x.astype(bf16)
    xpad = np.zeros((NPAD, D), np.float32)
    xpad[pos[:N_NODES]] = x
    W1a = np.concatenate([np.asarray(W1_loop, np.float32),
                          np.asarray(b1, np.float32)[None, :]], 0).astype(bf16)
    W2a = np.concatenate([np.asarray(W2_loop, np.float32),
                          np.asarray(b2, np.float32)[None, :]], 0).astype(bf16)
    in_maps = []
    for c in range(NC):
        xs = xpad[c * SHARD:(c + 1) * SHARD]
        xT_aug = np.ones((D + 1, SHARD), bf16)
        xT_aug[:D] = xs.T.astype(bf16)
        in_maps.append(dict(
            xtab=xtab,
            xT_aug=xT_aug,
            xT_half=np.ascontiguousarray(0.5 * xs.T).astype(np.float32),
            W1=np.asarray(W1, np.float32).astype(bf16),
            W2=np.asarray(W2, np.float32).astype(bf16),
            W1a=W1a, W2a=W2a,
            S=pp[c]["S"],
            idx_lo=pp[c]["idx_lo"],
            idx_hi=pp[c]["idx_hi"],
        ))
    return in_maps, layout


def build_program2(layout, repeat=0, ag_reps=1, nq=NQ, mbufs=6, s_bufs=3,
                   agg_bufs=3, parts="all", mhbufs=4, aux_bufs=3, p2_bufs=2,
                   tr_bufs=2, s_resident=False):
    CALL_BLK = layout.get("call_blk", 8)
    CALL_IDX = CALL_BLK * 128
    """v2 SPMD program: dense slot streams, static pair-list matmul schedule."""
    NQ = nq
    off = layout["off"]
    ncall = layout["ncall"]
    pairs = layout["pairs"]
    chunk_pairs = layout["chunk_pairs"]
    npair = layout["npair"]
    nc = bacc.Bacc("TRN2", target_bir_lowering=False, debug=False, num_devices=NC,
                   num_swdge_queues=NQ)

    xtab = nc.dram_tensor("xtab", [NPAD, ELEM], BF16, kind="ExternalInput")
    xT_aug = nc.dram_tensor("xT_aug", [D + 1, SHARD], BF16, kind="ExternalInput")
    xT_half = nc.dram_tensor("xT_half", [D, SHARD], F32, kind="ExternalInput")
    W1 = nc.dram_tensor("W1", [D, D], BF16, kind="ExternalInput")
    W2 = nc.dram_tensor("W2", [D, D], BF16, kind="ExternalInput")
    W1a = nc.dram_tensor("W1a", [D + 1, D], BF16, kind="ExternalInput")
    W2a = nc.dram_tensor("W2a", [D + 1, D], BF16, kind="ExternalInput")
    S_d = nc.dram_tensor("S", [128, 64 * npair], BF16, kind="ExternalInput")
    idx_lo_d = nc.dram_tensor("idx_lo", [ncall[0], 128, CALL_IDX // 16], I16,
                              kind="ExternalInput")
    idx_hi_d = nc.dram_tensor("idx_hi", [ncall[1], 128, CALL_IDX // 16], I16,
                              kind="ExternalInput")
    outT = nc.dram_tensor("outT", [D, SHARD], F32, kind="ExternalOutput")

    with tile.TileContext(nc) as tc:
        from contextlib import ExitStack
        with ExitStack() as ctx:
            const = ctx.enter_context(tc.tile_pool(name="const", bufs=1))
            idxp = ctx.enter_context(tc.tile_pool(name="idxp", bufs=mbufs))
            mlop = ctx.enter_context(tc.tile_pool(name="mlop", bufs=mbufs))
            mhip = ctx.enter_context(tc.tile_pool(name="mhip", bufs=mhbufs))
            sp = ctx.enter_context(tc.tile_pool(name="sp", bufs=s_bufs))
            srcp = ctx.enter_context(tc.tile_pool(name="srcp", bufs=aux_bufs))
            aggsbp = ctx.enter_context(tc.tile_pool(name="aggsbp", bufs=aux_bufs))
            rowp = ctx.enter_context(tc.tile_pool(name="rowp", bufs=aux_bufs))
            outp = ctx.enter_context(tc.tile_pool(name="outp", bufs=aux_bufs))
            aggps = ctx.enter_context(tc.tile_pool(name="aggps", bufs=agg_bufs,
                                                   space="PSUM"))
            p2ps = ctx.enter_context(tc.tile_pool(name="p2ps", bufs=p2_bufs,
                                                  space="PSUM"))
            trps = ctx.enter_context(tc.tile_pool(name="trps", bufs=tr_bufs,
                                                  space="PSUM"))

            nc.gpsimd.load_library(mlp)

            ident_d = nc.inline_tensor(np.eye(D, dtype=bf16), name="ident_bf16")
            ident = const.tile([D, D], BF16)
            nc.sync.dma_start(ident[:], ident_d.ap())
            w1 = const.tile([D, D], BF16)
            nc.sync.dma_start(w1[:], W1.ap())
            w2 = const.tile([D, D], BF16)
            nc.sync.dma_start(w2[:], W2.ap())
            w1a = const.tile([D + 1, D], BF16)
            nc.sync.dma_start(w1a[:], W1a.ap())
            w2a = const.tile([D + 1, D], BF16)
            nc.sync.dma_start(w2a[:], W2a.ap())

            h1t = const.tile([D + 1, SHARD], BF16)
            nc.vector.memset(h1t[D:D + 1, :], 1.0)

            S_sb = None
            if s_resident:
                # whole selector matrix stays in SBUF for both layers
                # (edge weights are layer-invariant); sliced loads so early
                # chunks' matmuls don't wait on the full 16.5MB transfer
                S_sb = const.tile([128, 64 * npair], BF16)
                ssl = -(-npair // 13)
                for i in range(13):
                    a, b = 64 * i * ssl, 64 * min((i + 1) * ssl, npair)
                    if a < b:
                        nc.sync.dma_start(S_sb[:, a:b], S_d.ap()[:, a:b])

            state = {"gq": 0, "prev_gather": None}
            h1_local = nc.dram_tensor("h1_local", [SHARD, ELEM], BF16,
                                      kind="Internal").ap()
            h1_table = nc.dram_tensor("h1_table", [NPAD, ELEM], BF16, kind="Internal",
                                      addr_space="Shared").ap()

            GRP = 8                       # chunks per grouped aux DMA
            NGRP = -(-NCHUNK // GRP)

            def gconv(layer, table_ap, w_t, wa_t):
                call_tiles = [{}, {}]

                def emit_call(t, idx_d, ci):
                    it = idxp.tile([128, CALL_IDX // 16], I16, tag=f"it{t}")
                    nc.sync.dma_start(it[:], idx_d.ap()[ci])
                    m = (mlop if t == 0 else mhip).tile(
                        [128, CALL_BLK, ELEM], BF16, tag=f"m{t}")
                    base = table_ap[0:HALF, :] if t == 0 else table_ap[HALF:NPAD, :]
                    if parts == "nogather":
                        nc.vector.memset(m[:, 0:1, :], 0.0)
                        call_tiles[t][ci] = m
                        return
                    gi = nc.gpsimd.dma_gather(m[:], base, it[:], CALL_IDX, CALL_IDX,
                                              ELEM, queue_num=state["gq"] % NQ)
                    state["gq"] += 1
                    if state["prev_gather"] is not None:
                        add_dep_helper(gi.ins, state["prev_gather"].ins, sync=False,
                                       reason="swdge queue/sem-lane consistency")
                    state["prev_gather"] = gi
                    call_tiles[t][ci] = m

                grp_state = {}

                gmax = max(
                    chunk_pairs[min((g + 1) * GRP, NCHUNK) - 1][0]
                    + chunk_pairs[min((g + 1) * GRP, NCHUNK) - 1][1]
                    - chunk_pairs[g * GRP][0]
                    for g in range(NGRP))

                def grp_begin(g):
                    k0, k1 = g * GRP, min((g + 1) * GRP, NCHUNK)
                    p0 = chunk_pairs[k0][0]
                    p1 = chunk_pairs[k1 - 1][0] + chunk_pairs[k1 - 1][1]
                    w = k1 - k0
                    if s_resident:
                        st = None
                    else:
                        st = sp.tile([128, 64 * gmax], BF16, tag="st")
                        nc.sync.dma_start(st[:, :64 * (p1 - p0)],
                                          S_d.ap()[:, 64 * p0:64 * p1])
                    grp_state.update(st=st, p0g=p0, k0=k0, w=w)
                    if layer == 1:
                        src = srcp.tile([D + 1, GRP * CHUNK], BF16, tag="src")
                        nc.sync.dma_start(src[:, :w * CHUNK],
                                          xT_aug.ap()[:, k0 * CHUNK:k1 * CHUNK])
                        grp_state["src"] = src
                    else:
                        xh = srcp.tile([D, GRP * CHUNK], F32, tag="xh")
                        nc.sync.dma_start(xh[:, :w * CHUNK],
                                          xT_half.ap()[:, k0 * CHUNK:k1 * CHUNK])
                        grp_state["xh"] = xh
                        ot = outp.tile([D, GRP * CHUNK], F32, tag="ot")
                        grp_state["ot"] = ot

                def grp_end(g):
                    k0, k1 = g * GRP, min((g + 1) * GRP, NCHUNK)
                    if layer == 2:
                        nc.sync.dma_start(outT.ap()[:, k0 * CHUNK:k1 * CHUNK],
                                          grp_state["ot"][:, :(k1 - k0) * CHUNK])

                for k in range(NCHUNK):
                    p0, npk = chunk_pairs[k]
                    for t, b, _ in pairs[p0:p0 + npk]:
                        ci = b // CALL_BLK
                        if ci not in call_tiles[t]:
                            emit_call(t, idx_lo_d if t == 0 else idx_hi_d, ci)
                    if parts == "gather":
                        continue
                    if k % GRP == 0:
                        grp_begin(k // GRP)
                    kk = (k - grp_state["k0"]) * CHUNK
                    agg = aggps.tile([D, CHUNK], F32, tag="agg")
                    for j, (t, b, _) in enumerate(pairs[p0:p0 + npk]):
                        if s_resident:
                            sap = S_sb[:, 64 * (p0 + j):64 * (p0 + j + 1)]
                        else:
                            j0 = p0 - grp_state["p0g"]
                            sap = grp_state["st"][:, 64 * (j0 + j):64 * (j0 + j + 1)]
                        nc.tensor.matmul(
                            agg[:], call_tiles[t][b // CALL_BLK][:, b % CALL_BLK, 0:D],
                            sap,
                            start=(j == 0), stop=(j == npk - 1),
                            skip_group_check=True)
                    aggb = aggsbp.tile([D, CHUNK], BF16, tag="aggb")
                    nc.scalar.activation(aggb[:], agg[:],
                                         mybir.ActivationFunctionType.Copy)
                    p2 = p2ps.tile([D, CHUNK], F32, tag="p2")
                    if layer == 1:
                        srcap = grp_state["src"][:, kk:kk + CHUNK]
                    else:
                        srcap = h1t[:, k * CHUNK:(k + 1) * CHUNK]
                    nc.tensor.matmul(p2[:], wa_t[:], srcap,
                                     start=True, stop=False, skip_group_check=True)
                    nc.tensor.matmul(p2[:], w_t[:], aggb[:],
                                     start=False, stop=True, skip_group_check=True)
                    if layer == 1:
                        hslice = h1t[0:D, k * CHUNK:(k + 1) * CHUNK]
                        nc.scalar.activation(hslice, p2[:],
                                             mybir.ActivationFunctionType.Relu)
                        trp = trps.tile([CHUNK, D], BF16, tag="trp")
                        nc.tensor.transpose(trp[:], hslice, ident[:])
                        if k % 2 == 0:
                            row = rowp.tile([2 * CHUNK, D], BF16, tag="row")
                            grp_state["row"] = row
                        else:
                            row = grp_state["row"]
                        half = (k % 2) * CHUNK
                        nc.vector.tensor_copy(row[half:half + CHUNK, :], trp[:])
                        if k % 2 == 1 or k == NCHUNK - 1:
                            k0r = (k // 2) * 2 * CHUNK
                            rows = half + CHUNK
                            nc.sync.dma_start(h1_local[k0r:k0r + rows, 0:D],
                                              row[0:rows, :])
                    else:
                        relu = outp.tile([D, CHUNK], F32, tag="relu")
                        nc.scalar.activation(relu[:], p2[:],
                                             mybir.ActivationFunctionType.Relu, scale=0.5)
                        nc.vector.tensor_add(grp_state["ot"][:, kk:kk + CHUNK],
                                             relu[:], grp_state["xh"][:, kk:kk + CHUNK])
                    if k % GRP == GRP - 1 or k == NCHUNK - 1:
                        grp_end(k // GRP)

            def allgather():
                nc.gpsimd.collective_compute(
                    "AllGather", mybir.AluOpType.bypass,
                    ins=[h1_local[:]], outs=[h1_table[:]],
                    replica_groups=[list(range(NC))],
                )

            if repeat > 0:
                with tc.For_i(0, repeat, 1):
                    gconv(1, xtab.ap(), w1, w1a)
                state["prev_gather"] = None
                for _ in range(ag_reps):
                    allgather()
                with tc.For_i(0, repeat, 1):
                    gconv(2, h1_table[:], w2, w2a)
            else:
                gconv(1, xtab.ap(), w1, w1a)
                allgather()
                gconv(2, h1_table[:], w2, w2a)

    nc.compile()
    return nc


def build_program(B_lo, B_hi, repeat=0, ag_reps=1, parts="all", nq=NQ, elem=ELEM,
                  mlo_bufs=6, mhi_bufs=4, idx_bufs=6, s_bufs=3, agg_bufs=3):
    """Build the SPMD Bass program. repeat>0 wraps each gconv phase in a
    hardware For_i loop and emits the AllGather ag_reps times (timing only;
    collectives cannot sit inside hardware loops)."""
    ELEM = elem
    NQ = nq
    NB = B_lo + B_hi
    NCALL_LO = -(-(NCHUNK * B_lo) // CALL_BLK)
    NCALL_HI = -(-(NCHUNK * B_hi) // CALL_BLK)
    nc = bacc.Bacc("TRN2", target_bir_lowering=False, debug=False, num_devices=NC,
                   num_swdge_queues=NQ)

    xtab = nc.dram_tensor("xtab", [NPAD, ELEM], BF16, kind="ExternalInput")
    xT_aug = nc.dram_tensor("xT_aug", [D + 1, SHARD], BF16, kind="ExternalInput")
    xT_half = nc.dram_tensor("xT_half", [D, SHARD], F32, kind="ExternalInput")
    W1 = nc.dram_tensor("W1", [D, D], BF16, kind="ExternalInput")
    W2 = nc.dram_tensor("W2", [D, D], BF16, kind="ExternalInput")
    W1a = nc.dram_tensor("W1a", [D + 1, D], BF16, kind="ExternalInput")
    W2a = nc.dram_tensor("W2a", [D + 1, D], BF16, kind="ExternalInput")
    S_d = nc.dram_tensor("S", [NCHUNK, 128, NB * CHUNK], BF16, kind="ExternalInput")
    idx_lo_d = nc.dram_tensor("idx_lo", [NCALL_LO, 128, CALL_IDX // 16], I16,
                              kind="ExternalInput")
    idx_hi_d = nc.dram_tensor("idx_hi", [NCALL_HI, 128, CALL_IDX // 16], I16,
                              kind="ExternalInput")
    outT = nc.dram_tensor("outT", [D, SHARD], F32, kind="ExternalOutput")

    with tile.TileContext(nc) as tc:
        from contextlib import ExitStack
        with ExitStack() as ctx:
            const = ctx.enter_context(tc.tile_pool(name="const", bufs=1))
            idxp = ctx.enter_context(tc.tile_pool(name="idxp", bufs=idx_bufs))
            mlop = ctx.enter_context(tc.tile_pool(name="mlop", bufs=mlo_bufs))
            mhip = ctx.enter_context(tc.tile_pool(name="mhip", bufs=mhi_bufs))
            sp = ctx.enter_context(tc.tile_pool(name="sp", bufs=s_bufs))
            srcp = ctx.enter_context(tc.tile_pool(name="srcp", bufs=3))
            aggsbp = ctx.enter_context(tc.tile_pool(name="aggsbp", bufs=3))
            rowp = ctx.enter_context(tc.tile_pool(name="rowp", bufs=3))
            outp = ctx.enter_context(tc.tile_pool(name="outp", bufs=3))
            aggps = ctx.enter_context(tc.tile_pool(name="aggps", bufs=agg_bufs, space="PSUM"))
            p2ps = ctx.enter_context(tc.tile_pool(name="p2ps", bufs=2, space="PSUM"))
            trps = ctx.enter_context(tc.tile_pool(name="trps", bufs=2, space="PSUM"))

            nc.gpsimd.load_library(mlp)

            ident_d = nc.inline_tensor(np.eye(D, dtype=bf16), name="ident_bf16")
            ident = const.tile([D, D], BF16)
            nc.sync.dma_start(ident[:], ident_d.ap())
            w1 = const.tile([D, D], BF16)
            nc.sync.dma_start(w1[:], W1.ap())
            w2 = const.tile([D, D], BF16)
            nc.sync.dma_start(w2[:], W2.ap())
            w1a = const.tile([D + 1, D], BF16)
            nc.sync.dma_start(w1a[:], W1a.ap())
            w2a = const.tile([D + 1, D], BF16)
            nc.sync.dma_start(w2a[:], W2a.ap())

            h1t = const.tile([D + 1, SHARD], BF16)   # persistent h1^T (+ones row)
            nc.vector.memset(h1t[D:D + 1, :], 1.0)

            state = {"gq": 0, "prev_gather": None}
            h1_local = nc.dram_tensor("h1_local", [SHARD, ELEM], BF16, kind="Internal").ap()
            h1_table = nc.dram_tensor("h1_table", [NPAD, ELEM], BF16, kind="Internal",
                                      addr_space="Shared").ap()

            def gconv(layer, table_ap, w_t, wa_t):
                lo_tiles = {}
                hi_tiles = {}

                NOHI = os.environ.get("GK_NOHI", "0") == "1"
                NOIDX = os.environ.get("GK_NOIDX", "0") == "1"

                def emit_call(tiles, idx_d, c, half):
                    if half == 1 and NOHI:
                        tiles[c] = None
                        return
                    it = idxp.tile([128, CALL_IDX // 16], I16, tag="it")
                    nc.sync.dma_start(it[:], idx_d.ap()[0] if NOIDX else idx_d.ap()[c])
                    m = (mlop if half == 0 else mhip).tile(
                        [128, CALL_BLK, ELEM], BF16, tag="m")
                    base = table_ap[0:HALF, :] if half == 0 else table_ap[HALF:NPAD, :]
                    if parts == "nogather":
                        nc.vector.memset(m[:, 0:1, :], 0.0)
                        tiles[c] = m
                        return
                    gi = nc.gpsimd.dma_gather(m[:], base, it[:], CALL_IDX, CALL_IDX,
                                              ELEM, queue_num=state["gq"] % NQ)
                    state["gq"] += 1
                    if state["prev_gather"] is not None:
                        # Keep Pool-engine order = emission order so Tile's
                        # 8-lane DMASW sem rotation stays aligned with the
                        # 4-queue rotation (sems are queue-locked).
                        add_dep_helper(gi.ins, state["prev_gather"].ins, sync=False,
                                       reason="swdge queue/sem-lane consistency")
                    state["prev_gather"] = gi
                    tiles[c] = m

                for k in range(NCHUNK):
                    for j in range(B_lo):
                        c = (k * B_lo + j) // CALL_BLK
                        if c not in lo_tiles:
                            emit_call(lo_tiles, idx_lo_d, c, 0)
                    for j in range(B_hi):
                        c = (k * B_hi + j) // CALL_BLK
                        if c not in hi_tiles:
                            emit_call(hi_tiles, idx_hi_d, c, 1)
                    if parts in ("gather", "g1"):
                        continue
                    st = sp.tile([128, NB * CHUNK], BF16, tag="st")
                    nc.sync.dma_start(st[:], S_d.ap()[k])
                    agg = aggps.tile([D, CHUNK], F32, tag="agg")
                    for j in range(B_lo):
                        b = k * B_lo + j
                        nc.tensor.matmul(
                            agg[:], lo_tiles[b // CALL_BLK][:, b % CALL_BLK, 0:D],
                            st[:, j * CHUNK:(j + 1) * CHUNK],
                            start=(j == 0), stop=False, skip_group_check=True)
                    for j in range(B_hi):
                        b = k * B_hi + j
                        nc.tensor.matmul(
                            agg[:], hi_tiles[b // CALL_BLK][:, b % CALL_BLK, 0:D],
                            st[:, (B_lo + j) * CHUNK:(B_lo + j + 1) * CHUNK],
                            start=False, stop=(j == B_hi - 1), skip_group_check=True)
                    aggb = aggsbp.tile([D, CHUNK], BF16, tag="aggb")
                    nc.scalar.activation(aggb[:], agg[:],
                                         mybir.ActivationFunctionType.Copy)
                    p2 = p2ps.tile([D, CHUNK], F32, tag="p2")
                    if layer == 1:
                        src = srcp.tile([D + 1, CHUNK], BF16, tag="src")
                        nc.sync.dma_start(src[:], xT_aug.ap()[:, k * CHUNK:(k + 1) * CHUNK])
                        srcap = src[:]
                    else:
                        srcap = h1t[:, k * CHUNK:(k + 1) * CHUNK]
                    nc.tensor.matmul(p2[:], wa_t[:], srcap,
                                     start=True, stop=False, skip_group_check=True)
                    nc.tensor.matmul(p2[:], w_t[:], aggb[:],
                                     start=False, stop=True, skip_group_check=True)
                    if layer == 1:
                        hslice = h1t[0:D, k * CHUNK:(k + 1) * CHUNK]
                        nc.scalar.activation(hslice, p2[:],
                                             mybir.ActivationFunctionType.Relu)
                        trp = trps.tile([CHUNK, D], BF16, tag="trp")
                        nc.tensor.transpose(trp[:], hslice, ident[:])
                        row = rowp.tile([CHUNK, D], BF16, tag="row")
                        nc.vector.tensor_copy(row[:], trp[:])
                        nc.sync.dma_start(h1_local[k * CHUNK:(k + 1) * CHUNK, 0:D], row[:])
                    else:
                        relu = outp.tile([D, CHUNK], F32, tag="relu")
                        nc.scalar.activation(relu[:], p2[:],
                                             mybir.ActivationFunctionType.Relu, scale=0.5)
                        xh = srcp.tile([D, CHUNK], F32, tag="xh")
                        nc.sync.dma_start(xh[:], xT_half.ap()[:, k * CHUNK:(k + 1) * CHUNK])
                        ot = outp.tile([D, CHUNK], F32, tag="ot")
                        nc.vector.tensor_add(ot[:], relu[:], xh[:])
                        nc.sync.dma_start(outT.ap()[:, k * CHUNK:(k + 1) * CHUNK], ot[:])

            if parts == "gather":
                zt = outp.tile([D, SHARD], F32, tag="zt")
                nc.vector.memset(zt[:], 0.0)
                nc.sync.dma_start(outT.ap(), zt[:])

            def allgather():
                nc.gpsimd.collective_compute(
                    "AllGather", mybir.AluOpType.bypass,
                    ins=[h1_local[:]], outs=[h1_table[:]],
                    replica_groups=[list(range(NC))],
                )

            if repeat > 0 and parts == "g1":
                with tc.For_i(0, repeat, 1):
                    gconv(1, xtab.ap(), w1, w1a)
                zt = outp.tile([D, SHARD], F32, tag="zt2")
                nc.vector.memset(zt[:], 0.0)
                nc.sync.dma_start(outT.ap(), zt[:])
            elif repeat > 0:
                with tc.For_i(0, repeat, 1):
                    gconv(1, xtab.ap(), w1, w1a)
                state["prev_gather"] = None
                for _ in range(ag_reps):
                    allgather()
                with tc.For_i(0, repeat, 1):
                    gconv(2, h1_table[:], w2, w2a)
            else:
                gconv(1, xtab.ap(), w1, w1a)
                allgather()
                gconv(2, h1_table[:], w2, w2a)

    nc.compile()
    return nc


_CACHE = {}

# best-measured pipeline configuration (sweeps 7-11): S resident in SBUF,
# 12 lo-call + 6 hi-call buffers, 4 PSUM agg banks
BEST_KW = dict(mbufs=12, mhbufs=6, agg_bufs=4, aux_bufs=4, s_resident=True)


def kernel(**inputs):
    if os.environ.get("GK_V1", "0") == "1":
        in_maps, B_lo, B_hi = _make_in_maps(**inputs)
        key = (B_lo, B_hi)
        if key not in _CACHE:
            _CACHE[key] = build_program(B_lo, B_hi)
        nc = _CACHE[key]
    else:
        in_maps, layout = _make_in_maps2(**inputs)
        key = (tuple(layout["ncall"]), layout["npair"],
               tuple(np.asarray(layout["off"]).ravel().tolist()))
        if key not in _CACHE:
            _CACHE[key] = build_program2(layout, **BEST_KW)
        nc = _CACHE[key]
        r = run_bass_kernel_spmd(nc, in_maps, list(range(NC)))
        out_cat = np.concatenate([r.results[c]["outT"].T for c in range(NC)], 0)
        out = out_cat[layout["pos"][:N_NODES]]
        return np.ascontiguousarray(out.astype(np.float32))
    r = run_bass_kernel_spmd(nc, in_maps, list(range(NC)))
    out = np.concatenate([r.results[c]["outT"].T for c in range(NC)], 0)[:N_NODES]
    return np.ascontiguousarray(out.astype(np.float32))



# revision 33
# speedup vs baseline: 33.8845x; 1.3668x over previous
"""Trainium2 Bass kernel for nn_GResBlock (2-layer weighted-GCN residual block).

    h1 = relu(A @ x @ W1 + x @ W1_loop + b1)
    h2 = relu(A @ h1 @ W2 + h1 @ W2_loop + b2)
    out = (x + h2) * 0.5
(A = 50000^2 sparse adjacency given as an 800000-edge weighted list.)

Strategy (8 NeuronCores, SPMD — one program, per-core data; "v2" path):
- Vertices padded to 50176 = 8*6272 rows. A host-side greedy permutation
  assigns dst nodes to (core, chunk-of-64, lane) bins with near-equal
  lo/hi in-edge counts (everything — x table, h1 table, output — lives in
  permuted order; the host inverse-permutes the final output).
- Aggregation is reordered as (A @ x) @ W (associativity), so the gather
  table for layer 1 is x itself (bf16, rows padded to 256B) — no support
  matrix is ever materialized.  Edges are split by src position < 32768
  (lo) / >= (hi) so int16 dma_gather indices stay in range (hi calls use
  a shifted base table AP).
- Dense slot streams: per stream, chunk k owns a core-invariant slot
  window of width max-over-cores count (~4% padding); 128-slot blocks and
  1024-index gather calls tile the stream ACROSS chunk boundaries
  (SWDGE desc-gen on the Q7 complex is the serial bottleneck at ~2.6ns
  per index slot, so slot count ≈ time; 1024 is a hard per-call cap —
  2048 wedges the device).  The matmul schedule is the static pair list
  (block, chunk) derived from the slot layout.
- Each pair is one PE matmul: stationary = gathered src rows [128, 96],
  moving = a host-built selector S [128, 64] (edge weight at the edge's
  dst lane; zeros elsewhere), accumulating agg^T [96, 64] f32 in PSUM.
  S is identical for both layers (same edge weights) and lives RESIDENT
  in SBUF (~129KB/partition) — no selector streaming in the loop.
- Then psum2 = Wloop_aug^T @ src_aug (bias folded via an ones row) +
  W^T @ agg -> relu -> h^T chunk.
- Layer 1 tail: PE-transpose each h1^T chunk; row pairs batched into
  [128, 96] stores to internal DRAM; one AllGather (~18us) builds the
  full layer-2 gather table.
- Layer 2 tail: out^T = x^T/2 + relu(psum2 * 0.5); per-8-chunk grouped
  xh loads and out stores (few big HWDGE DMAs — small per-chunk DMAs
  contend with the gather rings and stall the PE pipeline).
- Gathers spread round-robin over 4 SWDGE queues; explicit order deps
  keep Tile's DMASW sem-lane rotation consistent with the queue rotation.
"""
import os
import sys

import numpy as np
import ml_dtypes

try:
    import concourse.bass  # noqa: F401
except ImportError:
    sys.path.insert(0, "/opt/trn_rl_repo")

import concourse.bass as bass  # noqa: E402
import concourse.tile as tile  # noqa: E402
from concourse.tile_rust import add_dep_helper  # noqa: E402
from concourse import bacc, mybir  # noqa: E402
from concourse.library_config import mlp  # noqa: E402
from concourse.bass_utils import run_bass_kernel_spmd  # noqa: E402

bf16 = ml_dtypes.bfloat16
BF16 = mybir.dt.bfloat16
F32 = mybir.dt.float32
I16 = mybir.dt.int16

N_NODES = 50000
D = 96
NC = 8
SHARD = 6272
NPAD = NC * SHARD          # 50176
CHUNK = 64
NCHUNK = SHARD // CHUNK    # 98
HALF = 32768
ELEM = 128                 # gather element width (bf16 -> 256B)
NQ = 4                     # SWDGE queues
CALL_BLK = 8               # 128-edge blocks per gather call
CALL_IDX = 1024            # indices per gather call (hw scratch cap)


def _wrap_idx(idx):
    """[n] -> [128, n//16] int16 wrapped layout (idx i at [i%16, i//16],
    replicated across the 8 16-partition groups)."""
    n = idx.shape[0]
    w16 = idx.reshape(n // 16, 16).T.astype(np.int16)
    return np.tile(w16, (8, 1))


def _preprocess(edge_src, edge_dst, edge_weight, call_blk=CALL_BLK):
    CALL_BLK = call_blk
    CALL_IDX = call_blk * 128
    edge_src = np.asarray(edge_src).astype(np.int64)
    edge_dst = np.asarray(edge_dst).astype(np.int64)
    edge_weight = np.asarray(edge_weight).astype(np.float32)

    core_of = edge_dst // SHARD
    percore = []
    n_lo = np.zeros((NC, NCHUNK), np.int64)
    n_hi = np.zeros((NC, NCHUNK), np.int64)
    for c in range(NC):
        m = core_of == c
        s, d, w = edge_src[m], edge_dst[m], edge_weight[m]
        dl = d - c * SHARD
        ch = dl // CHUNK
        lane = dl % CHUNK
        lo = s < HALF
        order = np.lexsort((np.arange(len(s)), ~lo, ch))
        s, ch, lane, w, lo = s[order], ch[order], lane[order], w[order], lo[order]
        percore.append((s, ch, lane, w, lo))
        for k in range(NCHUNK):
            mk = ch == k
            n_lo[c, k] = np.count_nonzero(mk & lo)
            n_hi[c, k] = np.count_nonzero(mk & ~lo)

    B_lo = max(1, int(np.ceil(n_lo.max() / 128)))
    B_hi = max(1, int(np.ceil(n_hi.max() / 128)))
    NB = B_lo + B_hi

    out = []
    for c in range(NC):
        s, ch, lane, w, lo = percore[c]
        idx_lo = np.zeros((NCHUNK, B_lo * 128), np.int64)
        idx_hi = np.zeros((NCHUNK, B_hi * 128), np.int64)
        S = np.zeros((NCHUNK, 128, NB * CHUNK), np.float32)
        for k in range(NCHUNK):
            mk = ch == k
            sl, lal, wl, lol = s[mk], lane[mk], w[mk], lo[mk]
            a_s, a_l, a_w = sl[lol], lal[lol], wl[lol]
            nb = len(a_s)
            idx_lo[k, :nb] = a_s
            pos = np.arange(nb)
            S[k, pos % 128, (pos // 128) * CHUNK + a_l] = a_w
            b_s, b_l, b_w = sl[~lol], lal[~lol], wl[~lol]
            nb = len(b_s)
            idx_hi[k, :nb] = b_s - HALF
            pos = np.arange(nb)
            S[k, pos % 128, (B_lo + pos // 128) * CHUNK + b_l] = b_w

        def to_calls(idx2d, B):
            flat = idx2d.reshape(NCHUNK * B * 128)
            ncall = -(-(NCHUNK * B) // CALL_BLK)
            flat = np.concatenate([flat, np.zeros(ncall * CALL_IDX - flat.shape[0], np.int64)])
            return np.stack([_wrap_idx(flat[i * CALL_IDX:(i + 1) * CALL_IDX])
                             for i in range(ncall)])

        out.append(dict(
            idx_lo=to_calls(idx_lo, B_lo).astype(np.int16),
            idx_hi=to_calls(idx_hi, B_hi).astype(np.int16),
            S=S.astype(bf16),
        ))
    return out, B_lo, B_hi


def _make_in_maps(x, W1, W1_loop, b1, W2, W2_loop, b2, edge_weight, edge_src, edge_dst,
                  elem=ELEM, call_blk=CALL_BLK):
    ELEM = elem
    pp, B_lo, B_hi = _preprocess(edge_src, edge_dst, edge_weight, call_blk=call_blk)
    x = np.asarray(x, np.float32)
    xtab = np.zeros((NPAD, ELEM), bf16)
    xtab[:N_NODES, :D] = x.astype(bf16)
    xpad = np.zeros((NPAD, D), np.float32)
    xpad[:N_NODES] = x
    W1a = np.concatenate([np.asarray(W1_loop, np.float32),
                          np.asarray(b1, np.float32)[None, :]], 0).astype(bf16)
    W2a = np.concatenate([np.asarray(W2_loop, np.float32),
                          np.asarray(b2, np.float32)[None, :]], 0).astype(bf16)
    in_maps = []
    for c in range(NC):
        xs = xpad[c * SHARD:(c + 1) * SHARD]
        xT_aug = np.ones((D + 1, SHARD), bf16)
        xT_aug[:D] = xs.T.astype(bf16)
        in_maps.append(dict(
            xtab=xtab,
            xT_aug=xT_aug,
            xT_half=np.ascontiguousarray(0.5 * xs.T).astype(np.float32),
            W1=np.asarray(W1, np.float32).astype(bf16),
            W2=np.asarray(W2, np.float32).astype(bf16),
            W1a=W1a, W2a=W2a,
            S=pp[c]["S"],
            idx_lo=pp[c]["idx_lo"],
            idx_hi=pp[c]["idx_hi"],
        ))
    return in_maps, B_lo, B_hi


def _balance_perm(edge_src, edge_dst):
    """Assign dst nodes to (core, chunk, lane) slots so that every
    (core, chunk) bin has near-equal lo/hi in-edge counts.  Greedy
    largest-degree-first into the currently-lightest bin.  Returns pos[node]
    = permuted position.  Everything (x table, h1 table, output) uses
    pos-order so both layers share one stream structure."""
    nbin = NC * NCHUNK
    lo_deg = np.bincount(edge_dst[edge_src < HALF], minlength=N_NODES)[:N_NODES]
    hi_deg = np.bincount(edge_dst[edge_src >= HALF], minlength=N_NODES)[:N_NODES]
    tot = lo_deg + hi_deg
    order = np.argsort(-tot, kind="stable")
    lo_sum = np.zeros(nbin, np.int64)
    hi_sum = np.zeros(nbin, np.int64)
    n_in = np.zeros(nbin, np.int64)
    mu_lo = max(lo_deg.sum() / nbin, 1.0)
    mu_hi = max(hi_deg.sum() / nbin, 1.0)
    assign = np.empty(N_NODES, np.int64)
    # vectorized greedy: process nodes in batches of one per bin pass
    full = np.zeros(nbin, bool)
    for v in order:
        cost = np.maximum((lo_sum + lo_deg[v]) / mu_lo,
                          (hi_sum + hi_deg[v]) / mu_hi)
        cost[full] = np.inf
        b = int(np.argmin(cost))
        assign[v] = b
        lo_sum[b] += lo_deg[v]
        hi_sum[b] += hi_deg[v]
        n_in[b] += 1
        if n_in[b] == CHUNK:
            full[b] = True
    # positions: nodes of bin b occupy [b*CHUNK, b*CHUNK + n_in[b])
    pos = np.empty(NPAD, np.int64)
    fill = np.zeros(nbin, np.int64)
    for v in range(N_NODES):
        b = assign[v]
        pos[v] = b * CHUNK + fill[b]
        fill[b] += 1
    # pad nodes fill the remaining lanes
    spare = NPAD - N_NODES
    holes = []
    for b in range(nbin):
        holes.extend(range(b * CHUNK + fill[b], (b + 1) * CHUNK))
    assert len(holes) == spare
    pos[N_NODES:] = np.array(holes, np.int64)
    return pos


def _preprocess2(edge_src, edge_dst, edge_weight, balance=True,
                 call_blk=CALL_BLK):
    CALL_BLK = call_blk
    CALL_IDX = call_blk * 128
    """v2 layout: per stream (lo/hi), chunks occupy core-invariant slot
    windows of width max-over-cores count (~8% padding vs 25% for the v1
    per-chunk 128-rounded max).  128-slot blocks and 1024-slot gather calls
    tile each stream across chunk boundaries; the matmul schedule is the
    static pair list (block, chunk) derived from the slot layout.
    With balance=True, dst nodes are first permuted into (core, chunk) bins
    with near-equal counts, shrinking windows to ~the mean (#slots -> floor).
    NOTE: the lo/hi stream split keys on the PERMUTED src position."""
    edge_src = np.asarray(edge_src).astype(np.int64)
    edge_dst = np.asarray(edge_dst).astype(np.int64)
    edge_weight = np.asarray(edge_weight).astype(np.float32)
    if balance:
        pos = _balance_perm(edge_src, edge_dst)
        edge_src = pos[edge_src]
        edge_dst = pos[edge_dst]
    else:
        pos = np.arange(NPAD, dtype=np.int64)

    # per core / stream / chunk edge lists (stable order)
    percore = []
    cnt = np.zeros((NC, 2, NCHUNK), np.int64)
    for c in range(NC):
        m = (edge_dst // SHARD) == c
        s, d, w = edge_src[m], edge_dst[m], edge_weight[m]
        dl = d - c * SHARD
        ch = dl // CHUNK
        lane = dl % CHUNK
        hi = (s >= HALF).astype(np.int64)
        order = np.lexsort((np.arange(len(s)), hi, ch))
        s, ch, lane, w, hi = s[order], ch[order], lane[order], w[order], hi[order]
        percore.append((s, ch, lane, w, hi))
        for t in range(2):
            cnt[c, t] = np.bincount(ch[hi == t], minlength=NCHUNK)

    w_tk = cnt.max(axis=0)                     # [2, NCHUNK] slot window widths
    off = np.zeros((2, NCHUNK + 1), np.int64)  # cumulative slot offsets
    off[:, 1:] = np.cumsum(w_tk, axis=1)
    nslot = off[:, -1]
    ncall = [int(-(-nslot[t] // CALL_IDX)) for t in range(2)]
    nslot_pad = [ncall[t] * CALL_IDX for t in range(2)]

    # static matmul pair list: per chunk, lo pairs then hi pairs
    pairs = []            # (t, b, k)
    chunk_pairs = []      # per chunk: (start_pair_idx, npairs)
    for k in range(NCHUNK):
        p0 = len(pairs)
        for t in range(2):
            b0 = off[t, k] // 128
            b1 = (off[t, k + 1] - 1) // 128
            for b in range(b0, b1 + 1):
                pairs.append((t, b, k))
        chunk_pairs.append((p0, len(pairs) - p0))
    npair = len(pairs)

    out = []
    for c in range(NC):
        s, ch, lane, w, hi = percore[c]
        idx_flat = [np.zeros(nslot_pad[t], np.int64) for t in range(2)]
        S = np.zeros((128, 64 * npair), np.float32)
        # slot position of each edge: stream offset of its chunk + rank in chunk
        for t in range(2):
            mt = hi == t
            st_ch, st_lane, st_w, st_s = ch[mt], lane[mt], w[mt], s[mt]
            # rank within (chunk) in stable order
            rank = np.zeros(len(st_ch), np.int64)
            for k in range(NCHUNK):
                mk = st_ch == k
                rank[mk] = np.arange(np.count_nonzero(mk))
            slot = off[t, st_ch] + rank
            idx_flat[t][slot] = st_s - t * HALF
            blk = slot // 128
            # pair index of (t, blk, chunk): build lookup
            # pairs are ordered; map (t,b,k) -> pair idx
            pair_of = {pr: i for i, pr in enumerate(pairs)}
            pidx = np.array([pair_of[(t, int(b), int(k))]
                             for b, k in zip(blk, st_ch)], np.int64)
            S[slot % 128, 64 * pidx + st_lane] = st_w

        def to_calls(flat, n):
            return np.stack([_wrap_idx(flat[i * CALL_IDX:(i + 1) * CALL_IDX])
                             for i in range(n)])

        out.append(dict(
            idx_lo=to_calls(idx_flat[0], ncall[0]).astype(np.int16),
            idx_hi=to_calls(idx_flat[1], ncall[1]).astype(np.int16),
            S=S.astype(bf16),
        ))
    layout = dict(off=off, ncall=ncall, pairs=pairs, chunk_pairs=chunk_pairs,
                  npair=npair, pos=pos, call_blk=call_blk)
    return out, layout


def _make_in_maps2(x, W1, W1_loop, b1, W2, W2_loop, b2, edge_weight, edge_src,
                   edge_dst, call_blk=CALL_BLK):
    pp, layout = _preprocess2(edge_src, edge_dst, edge_weight, call_blk=call_blk)
    pos = layout["pos"]
    x = np.asarray(x, np.float32)
    xtab = np.zeros((NPAD, ELEM), bf16)
    xtab[pos[:N_NODES], :D] = x.astype(bf16)
    xpad = np.zeros((NPAD, D), np.float32)
    xpad[pos[:N_NODES]] = x
    W1a = np.concatenate([np.asarray(W1_loop, np.float32),
                          np.asarray(b1, np.float32)[None, :]], 0).astype(bf16)
    W2a = np.concatenate([np.asarray(W2_loop, np.float32),
                          np.asarray(b2, np.float32)[None, :]], 0).astype(bf16)
    in_maps = []
    for c in range(NC):
        xs = xpad[c * SHARD:(c + 1) * SHARD]
        xT_aug = np.ones((D + 1, SHARD), bf16)
        xT_aug[:D] = xs.T.astype(bf16)
        in_maps.append(dict(
            xtab=xtab,
            xT_aug=xT_aug,
            xT_half=np.ascontiguousarray(0.5 * xs.T).astype(np.float32),
            W1=np.asarray(W1, np.float32).astype(bf16),
            W2=np.asarray(W2, np.float32).astype(bf16),
            W1a=W1a, W2a=W2a,
            S=pp[c]["S"],
            idx_lo=pp[c]["idx_lo"],
            idx_hi=pp[c]["idx_hi"],
        ))
    return in_maps, layout


def build_program2(layout, repeat=0, ag_reps=1, nq=NQ, mbufs=6, s_bufs=3,
                   agg_bufs=3, parts="all", mhbufs=4, aux_bufs=3, p2_bufs=2,
                   tr_bufs=2, s_resident=False):
    CALL_BLK = layout.get("call_blk", 8)
    CALL_IDX = CALL_BLK * 128
    """v2 SPMD program: dense slot streams, static pair-list matmul schedule."""
    NQ = nq
    off = layout["off"]
    ncall = layout["ncall"]
    pairs = layout["pairs"]
    chunk_pairs = layout["chunk_pairs"]
    npair = layout["npair"]
    nc = bacc.Bacc("TRN2", target_bir_lowering=False, debug=False, num_devices=NC,
                   num_swdge_queues=NQ)

    xtab = nc.dram_tensor("xtab", [NPAD, ELEM], BF16, kind="ExternalInput")
    xT_aug = nc.dram_tensor("xT_aug", [D + 1, SHARD], BF16, kind="ExternalInput")
    xT_half = nc.dram_tensor("xT_half", [D, SHARD], F32, kind="ExternalInput")
    W1 = nc.dram_tensor("W1", [D, D], BF16, kind="ExternalInput")
    W2 = nc.dram_tensor("W2", [D, D], BF16, kind="ExternalInput")
    W1a = nc.dram_tensor("W1a", [D + 1, D], BF16, kind="ExternalInput")
    W2a = nc.dram_tensor("W2a", [D + 1, D], BF16, kind="ExternalInput")
    S_d = nc.dram_tensor("S", [128, 64 * npair], BF16, kind="ExternalInput")
    idx_lo_d = nc.dram_tensor("idx_lo", [ncall[0], 128, CALL_IDX // 16], I16,
                              kind="ExternalInput")
    idx_hi_d = nc.dram_tensor("idx_hi", [ncall[1], 128, CALL_IDX // 16], I16,
                              kind="ExternalInput")
    outT = nc.dram_tensor("outT", [D, SHARD], F32, kind="ExternalOutput")

    with tile.TileContext(nc) as tc:
        from contextlib import ExitStack
        with ExitStack() as ctx:
            const = ctx.enter_context(tc.tile_pool(name="const", bufs=1))
            idxp = ctx.enter_context(tc.tile_pool(name="idxp", bufs=mbufs))
            mlop = ctx.enter_context(tc.tile_pool(name="mlop", bufs=mbufs))
            mhip = ctx.enter_context(tc.tile_pool(name="mhip", bufs=mhbufs))
            sp = ctx.enter_context(tc.tile_pool(name="sp", bufs=s_bufs))
            srcp = ctx.enter_context(tc.tile_pool(name="srcp", bufs=aux_bufs))
            aggsbp = ctx.enter_context(tc.tile_pool(name="aggsbp", bufs=aux_bufs))
            rowp = ctx.enter_context(tc.tile_pool(name="rowp", bufs=aux_bufs))
            outp = ctx.enter_context(tc.tile_pool(name="outp", bufs=aux_bufs))
            aggps = ctx.enter_context(tc.tile_pool(name="aggps", bufs=agg_bufs,
                                                   space="PSUM"))
            p2ps = ctx.enter_context(tc.tile_pool(name="p2ps", bufs=p2_bufs,
                                                  space="PSUM"))
            trps = ctx.enter_context(tc.tile_pool(name="trps", bufs=tr_bufs,
                                                  space="PSUM"))

            nc.gpsimd.load_library(mlp)

            ident_d = nc.inline_tensor(np.eye(D, dtype=bf16), name="ident_bf16")
            ident = const.tile([D, D], BF16)
            nc.sync.dma_start(ident[:], ident_d.ap())
            w1 = const.tile([D, D], BF16)
            nc.sync.dma_start(w1[:], W1.ap())
            w2 = const.tile([D, D], BF16)
            nc.sync.dma_start(w2[:], W2.ap())
            w1a = const.tile([D + 1, D], BF16)
            nc.sync.dma_start(w1a[:], W1a.ap())
            w2a = const.tile([D + 1, D], BF16)
            nc.sync.dma_start(w2a[:], W2a.ap())

            h1t = const.tile([D + 1, SHARD], BF16)
            nc.vector.memset(h1t[D:D + 1, :], 1.0)

            S_sb = None
            if s_resident:
                # whole selector matrix stays in SBUF for both layers
                # (edge weights are layer-invariant); sliced loads so early
                # chunks' matmuls don't wait on the full 16.5MB transfer
                S_sb = const.tile([128, 64 * npair], BF16)
                ssl = -(-npair // 13)
                for i in range(13):
                    a, b = 64 * i * ssl, 64 * min((i + 1) * ssl, npair)
                    if a < b:
                        nc.sync.dma_start(S_sb[:, a:b], S_d.ap()[:, a:b])

            state = {"gq": 0, "prev_gather": None}
            h1_local = nc.dram_tensor("h1_local", [SHARD, ELEM], BF16,
                                      kind="Internal").ap()
            h1_table = nc.dram_tensor("h1_table", [NPAD, ELEM], BF16, kind="Internal",
                                      addr_space="Shared").ap()

            GRP = 8                       # chunks per grouped aux DMA
            NGRP = -(-NCHUNK // GRP)

            def gconv(layer, table_ap, w_t, wa_t):
                call_tiles = [{}, {}]

                def emit_call(t, idx_d, ci):
                    it = idxp.tile([128, CALL_IDX // 16], I16, tag=f"it{t}")
                    nc.sync.dma_start(it[:], idx_d.ap()[ci])
                    m = (mlop if t == 0 else mhip).tile(
                        [128, CALL_BLK, ELEM], BF16, tag=f"m{t}")
                    base = table_ap[0:HALF, :] if t == 0 else table_ap[HALF:NPAD, :]
                    if parts == "nogather":
                        nc.vector.memset(m[:, 0:1, :], 0.0)
                        call_tiles[t][ci] = m
                        return
                    gi = nc.gpsimd.dma_gather(m[:], base, it[:], CALL_IDX, CALL_IDX,
                                              ELEM, queue_num=state["gq"] % NQ)
                    state["gq"] += 1
                    if state["prev_gather"] is not None:
                        add_dep_helper(gi.ins, state["prev_gather"].ins, sync=False,
                                       reason="swdge queue/sem-lane consistency")
                    state["prev_gather"] = gi
                    call_tiles[t][ci] = m

                grp_state = {}

                gmax = max(
                    chunk_pairs[min((g + 1) * GRP, NCHUNK) - 1][0]
                    + chunk_pairs[min((g + 1) * GRP, NCHUNK) - 1][1]
                    - chunk_pairs[g * GRP][0]
                    for g in range(NGRP))

                def grp_begin(g):
                    k0, k1 = g * GRP, min((g + 1) * GRP, NCHUNK)
                    p0 = chunk_pairs[k0][0]
                    p1 = chunk_pairs[k1 - 1][0] + chunk_pairs[k1 - 1][1]
                    w = k1 - k0
                    if s_resident:
                        st = None
                    else:
                        st = sp.tile([128, 64 * gmax], BF16, tag="st")
                        nc.sync.dma_start(st[:, :64 * (p1 - p0)],
                                          S_d.ap()[:, 64 * p0:64 * p1])
                    grp_state.update(st=st, p0g=p0, k0=k0, w=w)
                    if layer == 1:
                        src = srcp.tile([D + 1, GRP * CHUNK], BF16, tag="src")
                        nc.sync.dma_start(src[:, :w * CHUNK],
                                          xT_aug.ap()[:, k0 * CHUNK:k1 * CHUNK])
                        grp_state["src"] = src
                    else:
                        xh = srcp.tile([D, GRP * CHUNK], F32, tag="xh")
                        nc.sync.dma_start(xh[:, :w * CHUNK],
                                          xT_half.ap()[:, k0 * CHUNK:k1 * CHUNK])
                        grp_state["xh"] = xh
                        ot = outp.tile([D, GRP * CHUNK], F32, tag="ot")
                        grp_state["ot"] = ot

                def grp_end(g):
                    k0, k1 = g * GRP, min((g + 1) * GRP, NCHUNK)
                    if layer == 2:
                        nc.sync.dma_start(outT.ap()[:, k0 * CHUNK:k1 * CHUNK],
                                          grp_state["ot"][:, :(k1 - k0) * CHUNK])

                for k in range(NCHUNK):
                    p0, npk = chunk_pairs[k]
                    for t, b, _ in pairs[p0:p0 + npk]:
                        ci = b // CALL_BLK
                        if ci not in call_tiles[t]:
                            emit_call(t, idx_lo_d if t == 0 else idx_hi_d, ci)
                    if parts == "gather":
                        continue
                    if k % GRP == 0:
                        grp_begin(k // GRP)
                    kk = (k - grp_state["k0"]) * CHUNK
                    agg = aggps.tile([D, CHUNK], F32, tag="agg")
                    for j, (t, b, _) in enumerate(pairs[p0:p0 + npk]):
                        if s_resident:
                            sap = S_sb[:, 64 * (p0 + j):64 * (p0 + j + 1)]
                        else:
                            j0 = p0 - grp_state["p0g"]
                            sap = grp_state["st"][:, 64 * (j0 + j):64 * (j0 + j + 1)]
                        nc.tensor.matmul(
                            agg[:], call_tiles[t][b // CALL_BLK][:, b % CALL_BLK, 0:D],
                            sap,
                            start=(j == 0), stop=(j == npk - 1),
                            skip_group_check=True)
                    aggb = aggsbp.tile([D, CHUNK], BF16, tag="aggb")
                    nc.scalar.activation(aggb[:], agg[:],
                                         mybir.ActivationFunctionType.Copy)
                    p2 = p2ps.tile([D, CHUNK], F32, tag="p2")
                    if layer == 1:
                        srcap = grp_state["src"][:, kk:kk + CHUNK]
                    else:
                        srcap = h1t[:, k * CHUNK:(k + 1) * CHUNK]
                    nc.tensor.matmul(p2[:], wa_t[:], srcap,
                                     start=True, stop=False, skip_group_check=True)
                    nc.tensor.matmul(p2[:], w_t[:], aggb[:],
                                     start=False, stop=True, skip_group_check=True)
                    if layer == 1:
                        hslice = h1t[0:D, k * CHUNK:(k + 1) * CHUNK]
                        nc.scalar.activation(hslice, p2[:],
                                             mybir.ActivationFunctionType.Relu)
                        trp = trps.tile([CHUNK, D], BF16, tag="trp")
                        nc.tensor.transpose(trp[:], hslice, ident[:])
                        if k % 2 == 0:
                            row = rowp.tile([2 * CHUNK, D], BF16, tag="row")
                            grp_state["row"] = row
                        else:
                            row = grp_state["row"]
                        half = (k % 2) * CHUNK
                        nc.vector.tensor_copy(row[half:half + CHUNK, :], trp[:])
                        if k % 2 == 1 or k == NCHUNK - 1:
                            k0r = (k // 2) * 2 * CHUNK
                            rows = half + CHUNK
                            nc.sync.dma_start(h1_local[k0r:k0r + rows, 0:D],
                                              row[0:rows, :])
                    else:
                        relu = outp.tile([D, CHUNK], F32, tag="relu")
                        nc.scalar.activation(relu[:], p2[:],
                                             mybir.ActivationFunctionType.Relu, scale=0.5)
                        nc.vector.tensor_add(grp_state["ot"][:, kk:kk + CHUNK],
                                             relu[:], grp_state["xh"][:, kk:kk + CHUNK])
                    if k % GRP == GRP - 1 or k == NCHUNK - 1:
                        grp_end(k // GRP)

            def allgather():
                nc.gpsimd.collective_compute(
                    "AllGather", mybir.AluOpType.bypass,
                    ins=[h1_local[:]], outs=[h1_table[:]],
                    replica_groups=[list(range(NC))],
                )

            if repeat > 0:
                with tc.For_i(0, repeat, 1):
                    gconv(1, xtab.ap(), w1, w1a)
                state["prev_gather"] = None
                for _ in range(ag_reps):
                    allgather()
                with tc.For_i(0, repeat, 1):
                    gconv(2, h1_table[:], w2, w2a)
            else:
                gconv(1, xtab.ap(), w1, w1a)
                allgather()
                gconv(2, h1_table[:], w2, w2a)

    nc.compile()
    return nc


def build_program(B_lo, B_hi, repeat=0, ag_reps=1, parts="all", nq=NQ, elem=ELEM,
                  mlo_bufs=6, mhi_bufs=4, idx_bufs=6, s_bufs=3, agg_bufs=3):
    """Build the SPMD Bass program. repeat>0 wraps each gconv phase in a
    hardware For_i loop and emits the AllGather ag_reps times (timing only;
    collectives cannot sit inside hardware loops)."""
    ELEM = elem
    NQ = nq
    NB = B_lo + B_hi
    NCALL_LO = -(-(NCHUNK * B_lo) // CALL_BLK)
    NCALL_HI = -(-(NCHUNK * B_hi) // CALL_BLK)
    nc = bacc.Bacc("TRN2", target_bir_lowering=False, debug=False, num_devices=NC,
                   num_swdge_queues=NQ)

    xtab = nc.dram_tensor("xtab", [NPAD, ELEM], BF16, kind="ExternalInput")
    xT_aug = nc.dram_tensor("xT_aug", [D + 1, SHARD], BF16, kind="ExternalInput")
    xT_half = nc.dram_tensor("xT_half", [D, SHARD], F32, kind="ExternalInput")
    W1 = nc.dram_tensor("W1", [D, D], BF16, kind="ExternalInput")
    W2 = nc.dram_tensor("W2", [D, D], BF16, kind="ExternalInput")
    W1a = nc.dram_tensor("W1a", [D + 1, D], BF16, kind="ExternalInput")
    W2a = nc.dram_tensor("W2a", [D + 1, D], BF16, kind="ExternalInput")
    S_d = nc.dram_tensor("S", [NCHUNK, 128, NB * CHUNK], BF16, kind="ExternalInput")
    idx_lo_d = nc.dram_tensor("idx_lo", [NCALL_LO, 128, CALL_IDX // 16], I16,
                              kind="ExternalInput")
    idx_hi_d = nc.dram_tensor("idx_hi", [NCALL_HI, 128, CALL_IDX // 16], I16,
                              kind="ExternalInput")
    outT = nc.dram_tensor("outT", [D, SHARD], F32, kind="ExternalOutput")

    with tile.TileContext(nc) as tc:
        from contextlib import ExitStack
        with ExitStack() as ctx:
            const = ctx.enter_context(tc.tile_pool(name="const", bufs=1))
            idxp = ctx.enter_context(tc.tile_pool(name="idxp", bufs=idx_bufs))
            mlop = ctx.enter_context(tc.tile_pool(name="mlop", bufs=mlo_bufs))
            mhip = ctx.enter_context(tc.tile_pool(name="mhip", bufs=mhi_bufs))
            sp = ctx.enter_context(tc.tile_pool(name="sp", bufs=s_bufs))
            srcp = ctx.enter_context(tc.tile_pool(name="srcp", bufs=3))
            aggsbp = ctx.enter_context(tc.tile_pool(name="aggsbp", bufs=3))
            rowp = ctx.enter_context(tc.tile_pool(name="rowp", bufs=3))
            outp = ctx.enter_context(tc.tile_pool(name="outp", bufs=3))
            aggps = ctx.enter_context(tc.tile_pool(name="aggps", bufs=agg_bufs, space="PSUM"))
            p2ps = ctx.enter_context(tc.tile_pool(name="p2ps", bufs=2, space="PSUM"))
            trps = ctx.enter_context(tc.tile_pool(name="trps", bufs=2, space="PSUM"))

            nc.gpsimd.load_library(mlp)

            ident_d = nc.inline_tensor(np.eye(D, dtype=bf16), name="ident_bf16")
            ident = const.tile([D, D], BF16)
            nc.sync.dma_start(ident[:], ident_d.ap())
            w1 = const.tile([D, D], BF16)
            nc.sync.dma_start(w1[:], W1.ap())
            w2 = const.tile([D, D], BF16)
            nc.sync.dma_start(w2[:], W2.ap())
            w1a = const.tile([D + 1, D], BF16)
            nc.sync.dma_start(w1a[:], W1a.ap())
            w2a = const.tile([D + 1, D], BF16)
            nc.sync.dma_start(w2a[:], W2a.ap())

            h1t = const.tile([D + 1, SHARD], BF16)   # persistent h1^T (+ones row)
            nc.vector.memset(h1t[D:D + 1, :], 1.0)

            state = {"gq": 0, "prev_gather": None}
            h1_local = nc.dram_tensor("h1_local", [SHARD, ELEM], BF16, kind="Internal").ap()
            h1_table = nc.dram_tensor("h1_table", [NPAD, ELEM], BF16, kind="Internal",
                                      addr_space="Shared").ap()

            def gconv(layer, table_ap, w_t, wa_t):
                lo_tiles = {}
                hi_tiles = {}

                NOHI = os.environ.get("GK_NOHI", "0") == "1"
                NOIDX = os.environ.get("GK_NOIDX", "0") == "1"

                def emit_call(tiles, idx_d, c, half):
                    if half == 1 and NOHI:
                        tiles[c] = None
                        return
                    it = idxp.tile([128, CALL_IDX // 16], I16, tag="it")
                    nc.sync.dma_start(it[:], idx_d.ap()[0] if NOIDX else idx_d.ap()[c])
                    m = (mlop if half == 0 else mhip).tile(
                        [128, CALL_BLK, ELEM], BF16, tag="m")
                    base = table_ap[0:HALF, :] if half == 0 else table_ap[HALF:NPAD, :]
                    if parts == "nogather":
                        nc.vector.memset(m[:, 0:1, :], 0.0)
                        tiles[c] = m
                        return
                    gi = nc.gpsimd.dma_gather(m[:], base, it[:], CALL_IDX, CALL_IDX,
                                              ELEM, queue_num=state["gq"] % NQ)
                    state["gq"] += 1
                    if state["prev_gather"] is not None:
                        # Keep Pool-engine order = emission order so Tile's
                        # 8-lane DMASW sem rotation stays aligned with the
                        # 4-queue rotation (sems are queue-locked).
                        add_dep_helper(gi.ins, state["prev_gather"].ins, sync=False,
                                       reason="swdge queue/sem-lane consistency")
                    state["prev_gather"] = gi
                    tiles[c] = m

                for k in range(NCHUNK):
                    for j in range(B_lo):
                        c = (k * B_lo + j) // CALL_BLK
                        if c not in lo_tiles:
                            emit_call(lo_tiles, idx_lo_d, c, 0)
                    for j in range(B_hi):
                        c = (k * B_hi + j) // CALL_BLK
                        if c not in hi_tiles:
                            emit_call(hi_tiles, idx_hi_d, c, 1)
                    if parts in ("gather", "g1"):
                        continue
                    st = sp.tile([128, NB * CHUNK], BF16, tag="st")
                    nc.sync.dma_start(st[:], S_d.ap()[k])
                    agg = aggps.tile([D, CHUNK], F32, tag="agg")
                    for j in range(B_lo):
                        b = k * B_lo + j
                        nc.tensor.matmul(
                            agg[:], lo_tiles[b // CALL_BLK][:, b % CALL_BLK, 0:D],
                            st[:, j * CHUNK:(j + 1) * CHUNK],
                            start=(j == 0), stop=False, skip_group_check=True)
                    for j in range(B_hi):
                        b = k * B_hi + j
                        nc.tensor.matmul(
                            agg[:], hi_tiles[b // CALL_BLK][:, b % CALL_BLK, 0:D],
                            st[:, (B_lo + j) * CHUNK:(B_lo + j + 1) * CHUNK],
                            start=False, stop=(j == B_hi - 1), skip_group_check=True)
                    aggb = aggsbp.tile([D, CHUNK], BF16, tag="aggb")
                    nc.scalar.activation(aggb[:], agg[:],
                                         mybir.ActivationFunctionType.Copy)
                    p2 = p2ps.tile([D, CHUNK], F32, tag="p2")
                    if layer == 1:
                        src = srcp.tile([D + 1, CHUNK], BF16, tag="src")
                        nc.sync.dma_start(src[:], xT_aug.ap()[:, k * CHUNK:(k + 1) * CHUNK])
                        srcap = src[:]
                    else:
                        srcap = h1t[:, k * CHUNK:(k + 1) * CHUNK]
                    nc.tensor.matmul(p2[:], wa_t[:], srcap,
                                     start=True, stop=False, skip_group_check=True)
                    nc.tensor.matmul(p2[:], w_t[:], aggb[:],
                                     start=False, stop=True, skip_group_check=True)
                    if layer == 1:
                        hslice = h1t[0:D, k * CHUNK:(k + 1) * CHUNK]
                        nc.scalar.activation(hslice, p2[:],
                                             mybir.ActivationFunctionType.Relu)
                        trp = trps.tile([CHUNK, D], BF16, tag="trp")
                        nc.tensor.transpose(trp[:], hslice, ident[:])
                        row = rowp.tile([CHUNK, D], BF16, tag="row")
                        nc.vector.tensor_copy(row[:], trp[:])
                        nc.sync.dma_start(h1_local[k * CHUNK:(k + 1) * CHUNK, 0:D], row[:])
                    else:
                        relu = outp.tile([D, CHUNK], F32, tag="relu")
                        nc.scalar.activation(relu[:], p2[:],
                                             mybir.ActivationFunctionType.Relu, scale=0.5)
                        xh = srcp.tile([D, CHUNK], F32, tag="xh")
                        nc.sync.dma_start(xh[:], xT_half.ap()[:, k * CHUNK:(k + 1) * CHUNK])
                        ot = outp.tile([D, CHUNK], F32, tag="ot")
                        nc.vector.tensor_add(ot[:], relu[:], xh[:])
                        nc.sync.dma_start(outT.ap()[:, k * CHUNK:(k + 1) * CHUNK], ot[:])

            if parts == "gather":
                zt = outp.tile([D, SHARD], F32, tag="zt")
                nc.vector.memset(zt[:], 0.0)
                nc.sync.dma_start(outT.ap(), zt[:])

            def allgather():
                nc.gpsimd.collective_compute(
                    "AllGather", mybir.AluOpType.bypass,
                    ins=[h1_local[:]], outs=[h1_table[:]],
                    replica_groups=[list(range(NC))],
                )

            if repeat > 0 and parts == "g1":
                with tc.For_i(0, repeat, 1):
                    gconv(1, xtab.ap(), w1, w1a)
                zt = outp.tile([D, SHARD], F32, tag="zt2")
                nc.vector.memset(zt[:], 0.0)
                nc.sync.dma_start(outT.ap(), zt[:])
            elif repeat > 0:
                with tc.For_i(0, repeat, 1):
                    gconv(1, xtab.ap(), w1, w1a)
                state["prev_gather"] = None
                for _ in range(ag_reps):
                    allgather()
                with tc.For_i(0, repeat, 1):
                    gconv(2, h1_table[:], w2, w2a)
            else:
                gconv(1, xtab.ap(), w1, w1a)
                allgather()
                gconv(2, h1_table[:], w2, w2a)

    nc.compile()
    return nc


_CACHE = {}

# best-measured pipeline configuration (sweeps 7-11): S resident in SBUF,
# 12 lo-call + 6 hi-call buffers, 4 PSUM agg banks
BEST_KW = dict(mbufs=12, mhbufs=6, agg_bufs=4, aux_bufs=4, s_resident=True)


def kernel(**inputs):
    if os.environ.get("GK_V1", "0") == "1":
        in_maps, B_lo, B_hi = _make_in_maps(**inputs)
        key = (B_lo, B_hi)
        if key not in _CACHE:
            _CACHE[key] = build_program(B_lo, B_hi)
        nc = _CACHE[key]
    else:
        in_maps, layout = _make_in_maps2(**inputs)
        key = (tuple(layout["ncall"]), layout["npair"],
               tuple(np.asarray(layout["off"]).ravel().tolist()))
        if key not in _CACHE:
            _CACHE[key] = build_program2(layout, **BEST_KW)
        nc = _CACHE[key]
        r = run_bass_kernel_spmd(nc, in_maps, list(range(NC)))
        out_cat = np.concatenate([r.results[c]["outT"].T for c in range(NC)], 0)
        out = out_cat[layout["pos"][:N_NODES]]
        return np.ascontiguousarray(out.astype(np.float32))
    r = run_bass_kernel_spmd(nc, in_maps, list(range(NC)))
    out = np.concatenate([r.results[c]["outT"].T for c in range(NC)], 0)[:N_NODES]
    return np.ascontiguousarray(out.astype(np.float32))

